# revision 11
# baseline (speedup 1.0000x reference)
"""Trainium2 Bass kernel: transformer block with dilated (parity-strided,
banded, causal) attention.

Problem: B=2, L=2048, E=768, H=12 heads, D=64, DILATION=2, WINDOW=256,
causal, pre-LN attention + pre-LN GELU FFN, fp32 reference.

Distribution (8 cores, no collectives): data-parallel over
(batch, sequence-chunk).  Core c handles batch c//4, tokens
[512*(c%4), 512*(c%4)+512).  Each core also receives the 256 preceding
tokens (halo) so the 256-window causal attention is fully local.

Key device-side ideas:
  * everything stays feature-major ([E, tokens] SBUF layout); the host
    pre-transposes x and the weights so no on-device transposes exist.
  * dilation folds away: query i attends key j only when (i-j)%2==0, so
    attention decomposes into two independent parity sequences, which are
    just stride-2 views of the token axis.  In folded coordinates the mask
    is a plain 129-wide causal band -> two 128x128 triangular-masked key
    blocks per 128-query block.
  * scores are computed transposed (S^T[k,q]) so no P^T transpose is needed
    for the P@V matmul; softmax runs without max-subtraction (scores are
    O(1)); the denominator comes from a ones-column appended to V; the
    per-query normalization scale is broadcast across partitions with a
    tiny ones-matmul.
  * LN statistics are computed with ones-vector matmuls over fp32r operands
    (full-rate, near-fp32 accuracy); LN gamma/beta are folded into the
    following projection weights on the host.
  * GEMMs run in bf16 with fp32 PSUM accumulation.
"""

import numpy as np
import ml_dtypes

import concourse.bass as bass
import concourse.bacc as bacc
import concourse.mybir as mybir
import concourse.tile as tile
from concourse.bass_utils import run_bass_kernel_spmd

BF16NP = ml_dtypes.bfloat16
F32 = mybir.dt.float32
F32R = mybir.dt.float32r
BF16 = mybir.dt.bfloat16
AF = mybir.ActivationFunctionType
OP = mybir.AluOpType

P = 128
B, L, E = 2, 2048, 768
ET = E // P            # 6 tiles over E
H, D = 12, 64
MLP = 4 * E            # 3072
MT = MLP // P          # 24 tiles over MLP hidden
OWN = 512              # tokens owned per core
HALO = 256             # preceding-context tokens per core
SLAB = OWN + HALO      # 768 tokens of x per core
EPS = 1e-5
N_CORES = 8

# head pairs that share one score/PV pipeline step; both heads of a pair sit
# on the same 64-partition row range of the feature-major layout so the
# normalized output can be scattered with a single strided DVE write.
PAIRS = [(0, 2), (4, 6), (8, 10), (1, 3), (5, 7), (9, 11)]


def _expand(apv, dim, count):
    """Insert a stride-0 dim of size `count` at free-dim position `dim`."""
    import dataclasses
    newap = [list(x) for x in apv.ap]
    newap.insert(dim, [0, count])
    return dataclasses.replace(apv, ap=newap)


def _fold2(apv):
    """[.., T] -> [.., 2, T//2] parity view of a stride-1 token axis."""
    return apv.rearrange("... (t two) -> ... two t", two=2)


def build_program():
    nc = bacc.Bacc("TRN2", target_bir_lowering=False, debug=False)

    xT = nc.dram_tensor("xT", [E, SLAB], F32, kind="ExternalInput").ap()
    qkv_wT = nc.dram_tensor("qkv_wT", [E, 3 * E], BF16, kind="ExternalInput").ap()
    out_wT = nc.dram_tensor("out_wT", [E, E], BF16, kind="ExternalInput").ap()
    ffn_w1T = nc.dram_tensor("ffn_w1T", [E, MLP], BF16, kind="ExternalInput").ap()
    ffn_w2T = nc.dram_tensor("ffn_w2T", [MLP, E], BF16, kind="ExternalInput").ap()
    qkv_b = nc.dram_tensor("qkv_b", [3 * E], F32, kind="ExternalInput").ap()
    out_b = nc.dram_tensor("out_b", [E], F32, kind="ExternalInput").ap()
    ffn_b1 = nc.dram_tensor("ffn_b1", [MLP], F32, kind="ExternalInput").ap()
    ffn_b2 = nc.dram_tensor("ffn_b2", [E], F32, kind="ExternalInput").ap()
    maskT = nc.dram_tensor("maskT", [2, 2, P, P], BF16, kind="ExternalInput").ap()
    yT = nc.dram_tensor("yT", [E, OWN], F32, kind="ExternalOutput").ap()

    with tile.TileContext(nc) as tc:
        _emit(tc, xT, qkv_wT, out_wT, ffn_w1T, ffn_w2T,
              qkv_b, out_b, ffn_b1, ffn_b2, maskT, yT)
    nc.compile()
    return nc


def _emit(tc, xT, qkv_wT, out_wT, ffn_w1T, ffn_w2T,
          qkv_b, out_b, ffn_b1, ffn_b2, maskT, yT):
    from contextlib import ExitStack
    ctx = ExitStack()
    nc = tc.nc

    sing = ctx.enter_context(tc.tile_pool(name="sing", bufs=1))
    wq_pool = ctx.enter_context(tc.tile_pool(name="wq", bufs=3))
    wv_pool = ctx.enter_context(tc.tile_pool(name="wv", bufs=3))
    w1_pool = ctx.enter_context(tc.tile_pool(name="w1", bufs=3))
    sq_pool = ctx.enter_context(tc.tile_pool(name="sq", bufs=2))  # [P,512] f32 transients
    ex_pool = ctx.enter_context(tc.tile_pool(name="ex", bufs=3))
    ot_pool = ctx.enter_context(tc.tile_pool(name="ot", bufs=2))
    row_pool = ctx.enter_context(tc.tile_pool(name="rows", bufs=4))
    rr_pool = ctx.enter_context(tc.tile_pool(name="rr", bufs=3))
    ft_pool = ctx.enter_context(tc.tile_pool(name="ftmp", bufs=2))

    ps_main = ctx.enter_context(tc.tile_pool(name="psg", bufs=2, space="PSUM"))
    ps_stats = ctx.enter_context(tc.tile_pool(name="psst", bufs=1, space="PSUM"))
    ps_attn = ctx.enter_context(tc.tile_pool(name="pssc", bufs=2, space="PSUM"))
    ps_pv = ctx.enter_context(tc.tile_pool(name="pspv", bufs=2, space="PSUM"))
    ps_bc = ctx.enter_context(tc.tile_pool(name="psbc", bufs=1, space="PSUM"))

    # ---------------- phase 0: input DMAs ----------------
    x_sb = sing.tile([P, ET, SLAB], F32, tag="x_sb")
    xT_v = xT.rearrange("(o p) t -> p o t", p=P)
    for et in range(ET):
        nc.sync.dma_start(out=x_sb[:, et, :], in_=xT_v[:, et, :])

    qkvb_sb = sing.tile([P, 18], F32, tag="qkvb")
    nc.sync.dma_start(out=qkvb_sb, in_=qkv_b.rearrange("(o p) -> p o", p=P))
    outb_sb = sing.tile([P, ET], F32, tag="outb")
    nc.sync.dma_start(out=outb_sb, in_=out_b.rearrange("(o p) -> p o", p=P))
    b1_sb = sing.tile([P, MT], F32, tag="b1")
    nc.sync.dma_start(out=b1_sb, in_=ffn_b1.rearrange("(o p) -> p o", p=P))
    b2_sb = sing.tile([P, ET], F32, tag="b2")
    nc.sync.dma_start(out=b2_sb, in_=ffn_b2.rearrange("(o p) -> p o", p=P))

    # masks replicated over the head-pair dim: [key, qb, h2, kb, q]
    masks_sb = sing.tile([P, 2, 2, 2, P], BF16, tag="masks")
    for qb in range(2):
        for hrep in range(2):
            for kb in range(2):
                nc.sync.dma_start(out=masks_sb[:, qb, hrep, kb, :],
                                  in_=maskT[qb, kb])

    outw_sb = sing.tile([P, ET, E], BF16, tag="outw")
    outw_v = out_wT.rearrange("(o p) e -> p o e", p=P)
    for et in range(ET):
        nc.sync.dma_start(out=outw_sb[:, et, :], in_=outw_v[:, et, :])
    w2_sb = sing.tile([P, MT, E], BF16, tag="w2")
    w2_v = ffn_w2T.rearrange("(o p) e -> p o e", p=P)
    for kt in range(MT):
        nc.sync.dma_start(out=w2_sb[:, kt, :], in_=w2_v[:, kt, :])

    ones_pf = sing.tile([P, 1], BF16, tag="ones_pf")
    nc.vector.memset(ones_pf, 1.0)
    ones_row = sing.tile([1, P], BF16, tag="ones_row")
    nc.vector.memset(ones_row, 1.0)
    eps_sb = sing.tile([1, 1], F32, tag="eps")
    nc.vector.memset(eps_sb, EPS)

    # ---------------- layernorm helper ----------------
    def emit_layernorm(src, dst, ntok):
        """dst = (src - mean)/sqrt(var+eps) over the E (partition x tile) axis.

        src: [P, ET, ntok] fp32, dst: [P, ET, ntok] bf16.  gamma/beta are
        folded into downstream weights on the host."""
        chunks = [(0, 512), (512, ntok - 512)] if ntok > 512 else [(0, ntok)]
        for c0, cl in chunks:
            st = ps_stats.tile([33, 512], F32, tag="st")
            for et in range(ET):
                xbf = sq_pool.tile([P, 512], BF16, tag="sqb")
                nc.vector.tensor_copy(out=xbf[:, :cl], in_=src[:, et, c0:c0 + cl])
                nc.tensor.matmul(st[0:1, :cl], ones_pf, xbf[:, :cl],
                                 start=(et == 0), stop=(et == ET - 1))
                xsq = sq_pool.tile([P, 512], BF16, tag="sq")
                nc.scalar.activation(xsq[:, :cl], src[:, et, c0:c0 + cl],
                                     AF.Square)
                nc.tensor.matmul(st[32:33, :cl], ones_pf, xsq[:, :cl],
                                 start=(et == 0), stop=(et == ET - 1))
            mu = row_pool.tile([1, 512], BF16, tag="rowb")
            nc.scalar.activation(mu[:, :cl], st[0:1, :cl], AF.Copy,
                                 scale=1.0 / E)
            musq = row_pool.tile([1, 512], F32, tag="row")
            nc.scalar.activation(musq[:, :cl], st[0:1, :cl], AF.Square,
                                 scale=1.0 / E)
            var = row_pool.tile([1, 512], F32, tag="row")
            nc.vector.scalar_tensor_tensor(
                out=var[:, :cl], in0=st[32:33, :cl], scalar=1.0 / E,
                in1=musq[:, :cl], op0=OP.mult, op1=OP.subtract)
            std = row_pool.tile([1, 512], F32, tag="row")
            nc.scalar.activation(std[:, :cl], var[:, :cl], AF.Sqrt, bias=eps_sb)
            a = row_pool.tile([1, 512], BF16, tag="rowb")
            with nc.allow_low_precision(reason="bf16 rstd broadcast operand"):
                nc.vector.reciprocal(a[:, :cl], std[:, :cl])

            murep = ps_bc.tile([P, 512], F32, tag="bc")
            nc.tensor.matmul(murep[:, :cl], ones_row, mu[:, :cl],
                             start=True, stop=True)
            for et in range(ET):
                nc.vector.tensor_sub(dst[:, et, c0:c0 + cl],
                                     src[:, et, c0:c0 + cl], murep[:, :cl])
            arep = ps_bc.tile([P, 512], F32, tag="bc")
            nc.tensor.matmul(arep[:, :cl], ones_row, a[:, :cl],
                             start=True, stop=True)
            for et in range(ET):
                nc.vector.tensor_mul(dst[:, et, c0:c0 + cl],
                                     dst[:, et, c0:c0 + cl], arep[:, :cl])

    # ---------------- phase 1: LN1 ----------------
    h_bf = sing.tile([P, ET, SLAB], BF16, tag="h_bf")
    emit_layernorm(x_sb, h_bf, SLAB)

    # ---------------- phase 2: QKV projections ----------------
    wq_view = qkv_wT.rearrange("(o p) f -> p o f", p=P)

    q_sb = sing.tile([P, ET, OWN], BF16, tag="q_sb")
    for ft in range(ET):
        wt = wq_pool.tile([P, ET, P], BF16, tag="wq")
        nc.sync.dma_start(out=wt, in_=wq_view[:, :, ft * P:(ft + 1) * P])
        ps = ps_main.tile([P, 512], F32, tag="g")
        for et in range(ET):
            nc.tensor.matmul(ps, wt[:, et, :], h_bf[:, et, HALO:SLAB],
                             start=(et == 0), stop=(et == ET - 1))
        nc.scalar.activation(q_sb[:, ft, :], ps, AF.Identity,
                             bias=qkvb_sb[:, ft:ft + 1])

    k_sb = sing.tile([P, ET, SLAB], BF16, tag="k_sb")
    for ft in range(ET):
        wt = wq_pool.tile([P, ET, P], BF16, tag="wq")
        nc.sync.dma_start(out=wt, in_=wq_view[:, :, E + ft * P:E + (ft + 1) * P])
        for c0, cl in [(0, 512), (512, 256)]:
            ps = ps_main.tile([P, 512], F32, tag="g")
            for et in range(ET):
                nc.tensor.matmul(ps[:, :cl], wt[:, et, :],
                                 h_bf[:, et, c0:c0 + cl],
                                 start=(et == 0), stop=(et == ET - 1))
            nc.scalar.activation(k_sb[:, ft, c0:c0 + cl], ps[:, :cl],
                                 AF.Identity, bias=qkvb_sb[:, 6 + ft:7 + ft])

    # V in [token, feature] orientation (tokens on partitions, folded blocks)
    # so the P@V matmul can contract over keys without transposes.  The V
    # bias is folded into out_b on the host (softmax rows sum to 1).
    v_sb = sing.tile([P, 2, 3, H, D + 1], BF16, tag="v_sb")
    nc.vector.memset(v_sb[:, :, :, :, D:D + 1], 1.0)
    for vc in range(3):                      # 256-wide chunks of V features
        wt = wv_pool.tile([P, ET, 256], BF16, tag="wv")
        nc.sync.dma_start(out=wt,
                          in_=wq_view[:, :, 2 * E + vc * 256:2 * E + (vc + 1) * 256])
        for par in range(2):
            for kb in range(3):
                ps = ps_main.tile([P, 512], F32, tag="g")
                for et in range(ET):
                    hblk = _fold2(h_bf[:, et, :])[:, par, kb * P:(kb + 1) * P]
                    nc.tensor.matmul(ps[:, :256], hblk, wt[:, et, :],
                                     start=(et == 0), stop=(et == ET - 1))
                nc.vector.tensor_copy(
                    out=v_sb[:, par, kb, 4 * vc:4 * vc + 4, 0:D],
                    in_=ps[:, :256].rearrange("p (h d) -> p h d", d=D))

    # ---------------- phase 3: dilated attention ----------------
    o_sb = sing.tile([P, ET, OWN], BF16, tag="o_sb")
    for par in range(2):
        for qb in range(2):
            for h0, h1 in PAIRS:
                kt = h0 // 2
                ro = D * (h0 % 2)
                sc = ps_attn.tile([P, 2, 2, P], F32, tag="sc")
                for hi, h in enumerate((h0, h1)):
                    ktt = h // 2
                    qv = _fold2(q_sb[ro:ro + D, ktt, :])[:, par,
                                                         qb * P:(qb + 1) * P]
                    kv = _fold2(k_sb[ro:ro + D, ktt, :])
                    for kbi, kb in enumerate((qb, qb + 1)):
                        nc.tensor.matmul(
                            sc[:, hi, kbi, :],
                            kv[:, par, kb * P:(kb + 1) * P], qv,
                            start=(hi == 0 and kbi == 0),
                            stop=(hi == 1 and kbi == 1))
                ex = ex_pool.tile([P, 2, 2, P], BF16, tag="ex")
                nc.scalar.activation(ex, sc, AF.Exp, scale=1.0 / np.sqrt(D))
                nc.vector.tensor_mul(ex, ex, masks_sb[:, qb])
                pv = ps_pv.tile([D + 1, 2, P], F32, tag="pv")
                for hi, h in enumerate((h0, h1)):
                    for kbi, kb in enumerate((qb, qb + 1)):
                        nc.tensor.matmul(
                            pv[:, hi, :], v_sb[:, par, kb, h, :],
                            ex[:, hi, kbi, :],
                            start=(hi == 0 and kbi == 0),
                            stop=(hi == 1 and kbi == 1))
                r = rr_pool.tile([1, 2, P], BF16, tag="r")
                with nc.allow_low_precision(reason="bf16 softmax-denominator broadcast"):
                    nc.vector.reciprocal(r, pv[D:D + 1, :, :])
                rrep = ps_bc.tile([P, 512], F32, tag="bc")
                nc.tensor.matmul(rrep[0:D, 0:256], ones_row[:, 0:D],
                                 r.rearrange("a b c -> a (b c)"),
                                 start=True, stop=True)
                ot = ot_pool.tile([D, 2, P], F32, tag="ot")
                nc.vector.tensor_copy(out=ot, in_=pv[0:D])
                dst = _fold2(o_sb[ro:ro + D, kt:kt + 2, :])[:, :, par,
                                                            qb * P:(qb + 1) * P]
                rv = rrep[0:D, 0:256].rearrange("p (x t) -> p x t", t=P)
                nc.vector.tensor_mul(dst, ot, rv)

    # ---------------- phase 4: out-proj + residual ----------------
    y1_sb = sing.tile([P, ET, OWN], F32, tag="y1_sb")
    for et in range(ET):
        ps = ps_main.tile([P, 512], F32, tag="g")
        for ftl in range(ET):
            nc.tensor.matmul(ps, outw_sb[:, ftl, et * P:(et + 1) * P],
                             o_sb[:, ftl, :],
                             start=(ftl == 0), stop=(ftl == ET - 1))
        t = ft_pool.tile([P, 512], F32, tag="ft")
        nc.scalar.activation(t, ps, AF.Identity, bias=outb_sb[:, et:et + 1])
        nc.vector.tensor_add(y1_sb[:, et, :], t, x_sb[:, et, HALO:SLAB])

    # ---------------- phase 5: LN2 ----------------
    h2_bf = sing.tile([P, ET, OWN], BF16, tag="h2_bf")
    emit_layernorm(y1_sb, h2_bf, OWN)

    # ---------------- phase 6: FFN1 + GELU ----------------
    w1_view = ffn_w1T.rearrange("(o p) f -> p o f", p=P)
    ffnh = sing.tile([P, MT, OWN], BF16, tag="ffnh")
    for mt in range(MT):
        wt = w1_pool.tile([P, ET, P], BF16, tag="w1")
        nc.sync.dma_start(out=wt, in_=w1_view[:, :, mt * P:(mt + 1) * P])
        ps = ps_main.tile([P, 512], F32, tag="g")
        for et in range(ET):
            nc.tensor.matmul(ps, wt[:, et, :], h2_bf[:, et, :],
                             start=(et == 0), stop=(et == ET - 1))
        nc.scalar.activation(ffnh[:, mt, :], ps, AF.Gelu,
                             bias=b1_sb[:, mt:mt + 1])

    # ---------------- phase 7: FFN2 + residual + store ----------------
    yT_view = yT.rearrange("(o p) t -> p o t", p=P)
    for et in range(ET):
        ps = ps_main.tile([P, 512], F32, tag="g")
        for ktl in range(MT):
            nc.tensor.matmul(ps, w2_sb[:, ktl, et * P:(et + 1) * P],
                             ffnh[:, ktl, :],
                             start=(ktl == 0), stop=(ktl == MT - 1))
        t = ft_pool.tile([P, 512], F32, tag="ft")
        nc.scalar.activation(t, ps, AF.Identity, bias=b2_sb[:, et:et + 1])
        nc.vector.tensor_add(y1_sb[:, et, :], t, y1_sb[:, et, :])
        nc.sync.dma_start(out=yT_view[:, et, :], in_=y1_sb[:, et, :])

    ctx.close()


# ======================= host side =======================

def prep_inputs(x, ln1_w, ln1_b, qkv_w, qkv_b, out_w, out_b,
                ln2_w, ln2_b, ffn_w1, ffn_b1, ffn_w2, ffn_b2):
    """Shard/fold/cast the full inputs into 8 per-core input maps."""
    x = np.asarray(x, np.float32)
    f8 = lambda v: np.asarray(v, np.float64)

    qkv_wT = (f8(qkv_w) * f8(ln1_w)[None, :]).T.astype(BF16NP).copy()
    qkv_b_eff = (f8(qkv_b) + f8(qkv_w) @ f8(ln1_b)).astype(np.float32)
    out_wT = f8(out_w).T.astype(BF16NP).copy()
    out_b_eff = (f8(out_b) + f8(out_w) @ f8(qkv_b)[2 * E:]).astype(np.float32)
    ffn_w1T = (f8(ffn_w1) * f8(ln2_w)[None, :]).T.astype(BF16NP).copy()
    ffn_b1_eff = (f8(ffn_b1) + f8(ffn_w1) @ f8(ln2_b)).astype(np.float32)
    ffn_w2T = f8(ffn_w2).T.astype(BF16NP).copy()
    ffn_b2_f = np.asarray(ffn_b2, np.float32)

    cidx = np.arange(P)[:, None]   # key (folded, within block)
    ridx = np.arange(P)[None, :]   # query (folded, within block)
    m_prev = (cidx >= ridx).astype(BF16NP)
    m_diag = (cidx <= ridx).astype(BF16NP)
    zero = np.zeros((P, P), BF16NP)

    in_maps = []
    for c in range(N_CORES):
        b, ch = divmod(c, 4)
        lo = OWN * ch - HALO
        if ch == 0:
            slab = np.concatenate(
                [np.zeros((HALO, E), np.float32), x[b, 0:OWN]], axis=0)
        else:
            slab = x[b, lo:lo + SLAB]
        xT = np.ascontiguousarray(slab.T)

        mask = np.stack([
            np.stack([zero if ch == 0 else m_prev, m_diag]),  # qb = 0
            np.stack([m_prev, m_diag]),                       # qb = 1
        ]).astype(BF16NP)

        in_maps.append({
            "xT": xT,
            "qkv_wT": qkv_wT, "out_wT": out_wT,
            "ffn_w1T": ffn_w1T, "ffn_w2T": ffn_w2T,
            "qkv_b": qkv_b_eff, "out_b": out_b_eff,
            "ffn_b1": ffn_b1_eff, "ffn_b2": ffn_b2_f,
            "maskT": np.ascontiguousarray(mask),
        })
    return in_maps


def gather_output(results):
    y = np.empty((B, L, E), np.float32)
    for c in range(N_CORES):
        b, ch = divmod(c, 4)
        y[b, OWN * ch:OWN * (ch + 1)] = results[c]["yT"].T
    return y


_NC_CACHE = None


def _get_program():
    global _NC_CACHE
    if _NC_CACHE is None:
        _NC_CACHE = build_program()
    return _NC_CACHE


def kernel(**inputs):
    nc = _get_program()
    in_maps = prep_inputs(**inputs)
    res = run_bass_kernel_spmd(nc, in_maps, core_ids=list(range(N_CORES)))
    return gather_output(res.results)


# revision 12
# speedup vs baseline: 1.1191x; 1.1191x over previous
"""Trainium2 Bass kernel: transformer block with dilated (parity-strided,
banded, causal) attention.

Problem: B=2, L=2048, E=768, H=12 heads, D=64, DILATION=2, WINDOW=256,
causal, pre-LN attention + pre-LN GELU FFN, fp32 reference.

Distribution (8 cores, no collectives): data-parallel over
(batch, sequence-chunk).  Core c handles batch c//4, tokens
[512*(c%4), 512*(c%4)+512).  Each core also receives the 256 preceding
tokens (halo) so the 256-window causal attention is fully local.

Key device-side ideas:
  * everything stays feature-major ([E, tokens] SBUF layout); the host
    pre-transposes x and the weights so no on-device transposes exist.
  * dilation folds away: query i attends key j only when (i-j)%2==0, so
    attention decomposes into two independent parity sequences, which are
    just stride-2 views of the token axis.  In folded coordinates the mask
    is a plain 129-wide causal band -> two 128x128 triangular-masked key
    blocks per 128-query block.
  * scores are computed transposed (S^T[k,q]) so no P^T transpose is needed
    for the P@V matmul; softmax runs without max-subtraction (scores are
    O(1)); the denominator comes from a ones-column appended to V; the
    per-query normalization scale is broadcast across partitions with a
    tiny ones-matmul.
  * LN statistics are computed with ones-vector matmuls over fp32r operands
    (full-rate, near-fp32 accuracy); LN gamma/beta are folded into the
    following projection weights on the host.
  * GEMMs run in bf16 with fp32 PSUM accumulation.
"""

import numpy as np
import ml_dtypes

import concourse.bass as bass
import concourse.bacc as bacc
import concourse.mybir as mybir
import concourse.tile as tile
from concourse.bass_utils import run_bass_kernel_spmd

BF16NP = ml_dtypes.bfloat16
F32 = mybir.dt.float32
F32R = mybir.dt.float32r
BF16 = mybir.dt.bfloat16
AF = mybir.ActivationFunctionType
OP = mybir.AluOpType

P = 128
B, L, E = 2, 2048, 768
ET = E // P            # 6 tiles over E
H, D = 12, 64
MLP = 4 * E            # 3072
MT = MLP // P          # 24 tiles over MLP hidden
OWN = 512              # tokens owned per core
HALO = 256             # preceding-context tokens per core
SLAB = OWN + HALO      # 768 tokens of x per core
EPS = 1e-5
N_CORES = 8

# head pairs that share one score/PV pipeline step; both heads of a pair sit
# on the same 64-partition row range of the feature-major layout so the
# normalized output can be scattered with a single strided DVE write.
PAIRS = [(0, 2), (4, 6), (8, 10), (1, 3), (5, 7), (9, 11)]


def _expand(apv, dim, count):
    """Insert a stride-0 dim of size `count` at free-dim position `dim`."""
    import dataclasses
    newap = [list(x) for x in apv.ap]
    newap.insert(dim, [0, count])
    return dataclasses.replace(apv, ap=newap)


def _fold2(apv):
    """[.., T] -> [.., 2, T//2] parity view of a stride-1 token axis."""
    return apv.rearrange("... (t two) -> ... two t", two=2)


def build_program():
    nc = bacc.Bacc("TRN2", target_bir_lowering=False, debug=False)

    xT = nc.dram_tensor("xT", [E, SLAB], F32, kind="ExternalInput").ap()
    qkv_wT = nc.dram_tensor("qkv_wT", [E, 3 * E], BF16, kind="ExternalInput").ap()
    out_wT = nc.dram_tensor("out_wT", [E, E], BF16, kind="ExternalInput").ap()
    ffn_w1T = nc.dram_tensor("ffn_w1T", [E, MLP], BF16, kind="ExternalInput").ap()
    ffn_w2T = nc.dram_tensor("ffn_w2T", [MLP, E], BF16, kind="ExternalInput").ap()
    qkv_b = nc.dram_tensor("qkv_b", [3 * E], F32, kind="ExternalInput").ap()
    out_b = nc.dram_tensor("out_b", [E], F32, kind="ExternalInput").ap()
    ffn_b1 = nc.dram_tensor("ffn_b1", [MLP], F32, kind="ExternalInput").ap()
    ffn_b2 = nc.dram_tensor("ffn_b2", [E], F32, kind="ExternalInput").ap()
    maskT = nc.dram_tensor("maskT", [2, 2, P, P], BF16, kind="ExternalInput").ap()
    yT = nc.dram_tensor("yT", [E, OWN], F32, kind="ExternalOutput").ap()

    with tile.TileContext(nc) as tc:
        _emit(tc, xT, qkv_wT, out_wT, ffn_w1T, ffn_w2T,
              qkv_b, out_b, ffn_b1, ffn_b2, maskT, yT)
    nc.compile()
    return nc


def _emit(tc, xT, qkv_wT, out_wT, ffn_w1T, ffn_w2T,
          qkv_b, out_b, ffn_b1, ffn_b2, maskT, yT):
    from contextlib import ExitStack
    ctx = ExitStack()
    nc = tc.nc

    sing = ctx.enter_context(tc.tile_pool(name="sing", bufs=1))
    wq_pool = ctx.enter_context(tc.tile_pool(name="wq", bufs=6))
    wv_pool = ctx.enter_context(tc.tile_pool(name="wv", bufs=2))
    w1_pool = ctx.enter_context(tc.tile_pool(name="w1", bufs=6))
    sq_pool = ctx.enter_context(tc.tile_pool(name="sq", bufs=2))  # [P,512] f32 transients
    ex_pool = ctx.enter_context(tc.tile_pool(name="ex", bufs=3))
    ot_pool = ctx.enter_context(tc.tile_pool(name="ot", bufs=2))
    row_pool = ctx.enter_context(tc.tile_pool(name="rows", bufs=4))
    rr_pool = ctx.enter_context(tc.tile_pool(name="rr", bufs=3))
    ft_pool = ctx.enter_context(tc.tile_pool(name="ftmp", bufs=2))

    ps_main = ctx.enter_context(tc.tile_pool(name="psg", bufs=2, space="PSUM"))
    ps_stats = ctx.enter_context(tc.tile_pool(name="psst", bufs=1, space="PSUM"))
    ps_attn = ctx.enter_context(tc.tile_pool(name="pssc", bufs=2, space="PSUM"))
    ps_pv = ctx.enter_context(tc.tile_pool(name="pspv", bufs=2, space="PSUM"))
    ps_bc = ctx.enter_context(tc.tile_pool(name="psbc", bufs=1, space="PSUM"))

    # ---------------- phase 0: input DMAs ----------------
    x_sb = sing.tile([P, ET, SLAB], F32, tag="x_sb")
    xT_v = xT.rearrange("(o p) t -> p o t", p=P)
    for et in range(ET):
        nc.sync.dma_start(out=x_sb[:, et, :], in_=xT_v[:, et, :])

    qkvb_sb = sing.tile([P, 18], F32, tag="qkvb")
    nc.sync.dma_start(out=qkvb_sb, in_=qkv_b.rearrange("(o p) -> p o", p=P))
    outb_sb = sing.tile([P, ET], F32, tag="outb")
    nc.sync.dma_start(out=outb_sb, in_=out_b.rearrange("(o p) -> p o", p=P))
    b1_sb = sing.tile([P, MT], F32, tag="b1")
    nc.sync.dma_start(out=b1_sb, in_=ffn_b1.rearrange("(o p) -> p o", p=P))
    b2_sb = sing.tile([P, ET], F32, tag="b2")
    nc.sync.dma_start(out=b2_sb, in_=ffn_b2.rearrange("(o p) -> p o", p=P))

    # masks replicated over the head-pair dim: [key, qb, h2, kb, q]
    masks_sb = sing.tile([P, 2, 2, 2, P], BF16, tag="masks")
    for qb in range(2):
        for hrep in range(2):
            for kb in range(2):
                nc.sync.dma_start(out=masks_sb[:, qb, hrep, kb, :],
                                  in_=maskT[qb, kb])


    ones_pf = sing.tile([P, 1], BF16, tag="ones_pf")
    nc.vector.memset(ones_pf, 1.0)
    ones_row = sing.tile([1, P], BF16, tag="ones_row")
    nc.vector.memset(ones_row, 1.0)
    eps_sb = sing.tile([1, 1], F32, tag="eps")
    nc.vector.memset(eps_sb, EPS)

    # ---------------- layernorm helper ----------------
    def emit_layernorm(src, dst, ntok):
        """dst = (src - mean)/sqrt(var+eps) over the E (partition x tile) axis.

        src: [P, ET, ntok] fp32, dst: [P, ET, ntok] bf16.  gamma/beta are
        folded into downstream weights on the host."""
        chunks = [(0, 512), (512, ntok - 512)] if ntok > 512 else [(0, ntok)]
        for c0, cl in chunks:
            st = ps_stats.tile([33, 512], F32, tag="st")
            for et in range(ET):
                xbf = sq_pool.tile([P, 512], BF16, tag="sqb")
                nc.vector.tensor_copy(out=xbf[:, :cl], in_=src[:, et, c0:c0 + cl])
                nc.tensor.matmul(st[0:1, :cl], ones_pf, xbf[:, :cl],
                                 start=(et == 0), stop=(et == ET - 1))
                xsq = sq_pool.tile([P, 512], BF16, tag="sq")
                nc.scalar.activation(xsq[:, :cl], src[:, et, c0:c0 + cl],
                                     AF.Square)
                nc.tensor.matmul(st[32:33, :cl], ones_pf, xsq[:, :cl],
                                 start=(et == 0), stop=(et == ET - 1))
            mu = row_pool.tile([1, 512], BF16, tag="rowb")
            nc.scalar.activation(mu[:, :cl], st[0:1, :cl], AF.Copy,
                                 scale=1.0 / E)
            musq = row_pool.tile([1, 512], F32, tag="row")
            nc.scalar.activation(musq[:, :cl], st[0:1, :cl], AF.Square,
                                 scale=1.0 / E)
            var = row_pool.tile([1, 512], F32, tag="row")
            nc.vector.scalar_tensor_tensor(
                out=var[:, :cl], in0=st[32:33, :cl], scalar=1.0 / E,
                in1=musq[:, :cl], op0=OP.mult, op1=OP.subtract)
            std = row_pool.tile([1, 512], F32, tag="row")
            nc.scalar.activation(std[:, :cl], var[:, :cl], AF.Sqrt, bias=eps_sb)
            a = row_pool.tile([1, 512], BF16, tag="rowb")
            with nc.allow_low_precision(reason="bf16 rstd broadcast operand"):
                nc.vector.reciprocal(a[:, :cl], std[:, :cl])

            murep = ps_bc.tile([P, 512], F32, tag="bc")
            nc.tensor.matmul(murep[:, :cl], ones_row, mu[:, :cl],
                             start=True, stop=True)
            for et in range(ET):
                nc.vector.tensor_sub(dst[:, et, c0:c0 + cl],
                                     src[:, et, c0:c0 + cl], murep[:, :cl])
            arep = ps_bc.tile([P, 512], F32, tag="bc")
            nc.tensor.matmul(arep[:, :cl], ones_row, a[:, :cl],
                             start=True, stop=True)
            for et in range(ET):
                nc.vector.tensor_mul(dst[:, et, c0:c0 + cl],
                                     dst[:, et, c0:c0 + cl], arep[:, :cl])

    # ---------------- phase 1: LN1 ----------------
    h_bf = sing.tile([P, ET, SLAB], BF16, tag="h_bf")
    emit_layernorm(x_sb, h_bf, SLAB)

    # ---------------- phase 2: QKV projections ----------------
    wq_view = qkv_wT.rearrange("(o p) f -> p o f", p=P)

    q_sb = sing.tile([P, ET, OWN], BF16, tag="q_sb")
    for ft in range(ET):
        wt = wq_pool.tile([P, ET, P], BF16, tag="wq")
        nc.sync.dma_start(out=wt, in_=wq_view[:, :, ft * P:(ft + 1) * P])
        ps = ps_main.tile([P, 512], F32, tag="g")
        for et in range(ET):
            nc.tensor.matmul(ps, wt[:, et, :], h_bf[:, et, HALO:SLAB],
                             start=(et == 0), stop=(et == ET - 1))
        nc.scalar.activation(q_sb[:, ft, :], ps, AF.Identity,
                             bias=qkvb_sb[:, ft:ft + 1])

    k_sb = sing.tile([P, ET, SLAB], BF16, tag="k_sb")
    for ft in range(ET):
        wt = wq_pool.tile([P, ET, P], BF16, tag="wq")
        nc.sync.dma_start(out=wt, in_=wq_view[:, :, E + ft * P:E + (ft + 1) * P])
        for c0, cl in [(0, 512), (512, 256)]:
            ps = ps_main.tile([P, 512], F32, tag="g")
            for et in range(ET):
                nc.tensor.matmul(ps[:, :cl], wt[:, et, :],
                                 h_bf[:, et, c0:c0 + cl],
                                 start=(et == 0), stop=(et == ET - 1))
            nc.scalar.activation(k_sb[:, ft, c0:c0 + cl], ps[:, :cl],
                                 AF.Identity, bias=qkvb_sb[:, 6 + ft:7 + ft])

    # V in [token, feature] orientation (tokens on partitions, folded blocks)
    # so the P@V matmul can contract over keys without transposes.  The V
    # bias is folded into out_b on the host (softmax rows sum to 1).
    v_sb = sing.tile([P, 2, 3, H, D + 1], BF16, tag="v_sb")
    nc.vector.memset(v_sb[:, :, :, :, D:D + 1], 1.0)
    for vc0, vcl in [(0, 512), (512, 256)]:   # V-feature chunks
        wt = wv_pool.tile([P, ET, 512], BF16, tag="wv")
        nc.sync.dma_start(out=wt[:, :, :vcl],
                          in_=wq_view[:, :, 2 * E + vc0:2 * E + vc0 + vcl])
        for par in range(2):
            for kb in range(3):
                ps = ps_main.tile([P, 512], F32, tag="g")
                for et in range(ET):
                    hblk = _fold2(h_bf[:, et, :])[:, par, kb * P:(kb + 1) * P]
                    nc.tensor.matmul(ps[:, :vcl], hblk, wt[:, et, :vcl],
                                     start=(et == 0), stop=(et == ET - 1))
                nc.vector.tensor_copy(
                    out=v_sb[:, par, kb, vc0 // D:(vc0 + vcl) // D, 0:D],
                    in_=ps[:, :vcl].rearrange("p (h d) -> p h d", d=D))

    # out-proj weights arrive during attention
    outw_sb = sing.tile([P, ET, E], BF16, tag="outw")
    outw_v = out_wT.rearrange("(o p) e -> p o e", p=P)
    for et in range(ET):
        nc.sync.dma_start(out=outw_sb[:, et, :], in_=outw_v[:, et, :])

    # ---------------- phase 3: dilated attention ----------------
    # pair-major order: o_sb feature tiles complete early so the out-proj
    # matmuls can fill attention-phase PE gaps.
    o_sb = sing.tile([P, ET, OWN], BF16, tag="o_sb")
    for h0, h1 in [(0, 2), (1, 3), (4, 6), (5, 7), (8, 10), (9, 11)]:
        for par in range(2):
            for qb in range(2):
                kt = h0 // 2
                ro = D * (h0 % 2)
                sc = ps_attn.tile([P, 2, 2, P], F32, tag="sc")
                for hi, h in enumerate((h0, h1)):
                    ktt = h // 2
                    qv = _fold2(q_sb[ro:ro + D, ktt, :])[:, par,
                                                         qb * P:(qb + 1) * P]
                    kv = _fold2(k_sb[ro:ro + D, ktt, :])
                    for kbi, kb in enumerate((qb, qb + 1)):
                        nc.tensor.matmul(
                            sc[:, hi, kbi, :],
                            kv[:, par, kb * P:(kb + 1) * P], qv,
                            start=(hi == 0 and kbi == 0),
                            stop=(hi == 1 and kbi == 1))
                ex = ex_pool.tile([P, 2, 2, P], BF16, tag="ex")
                nc.scalar.activation(ex, sc, AF.Exp, scale=1.0 / np.sqrt(D))
                nc.vector.tensor_mul(ex, ex, masks_sb[:, qb])
                pv = ps_pv.tile([D + 1, 2, P], F32, tag="pv")
                for hi, h in enumerate((h0, h1)):
                    for kbi, kb in enumerate((qb, qb + 1)):
                        nc.tensor.matmul(
                            pv[:, hi, :], v_sb[:, par, kb, h, :],
                            ex[:, hi, kbi, :],
                            start=(hi == 0 and kbi == 0),
                            stop=(hi == 1 and kbi == 1))
                r = rr_pool.tile([1, 2, P], BF16, tag="r")
                with nc.allow_low_precision(reason="bf16 softmax-denominator broadcast"):
                    nc.vector.reciprocal(r, pv[D:D + 1, :, :])
                rrep = ps_bc.tile([P, 512], F32, tag="bc")
                nc.tensor.matmul(rrep[0:D, 0:256], ones_row[:, 0:D],
                                 r.rearrange("a b c -> a (b c)"),
                                 start=True, stop=True)
                ot = ot_pool.tile([D, 2, P], F32, tag="ot")
                nc.vector.tensor_copy(out=ot, in_=pv[0:D])
                dst = _fold2(o_sb[ro:ro + D, kt:kt + 2, :])[:, :, par,
                                                            qb * P:(qb + 1) * P]
                rv = rrep[0:D, 0:256].rearrange("p (x t) -> p x t", t=P)
                nc.vector.tensor_mul(dst, ot, rv)

    # ---------------- phase 4: out-proj + residual ----------------
    y1_sb = sing.tile([P, ET, OWN], F32, tag="y1_sb")
    for et in range(ET):
        ps = ps_main.tile([P, 512], F32, tag="g")
        for ftl in range(ET):
            nc.tensor.matmul(ps, outw_sb[:, ftl, et * P:(et + 1) * P],
                             o_sb[:, ftl, :],
                             start=(ftl == 0), stop=(ftl == ET - 1))
        t = ft_pool.tile([P, 512], F32, tag="ft")
        nc.scalar.activation(t, ps, AF.Identity, bias=outb_sb[:, et:et + 1])
        nc.vector.tensor_add(y1_sb[:, et, :], t, x_sb[:, et, HALO:SLAB])

    # ---------------- phase 5: LN2 ----------------
    h2_bf = sing.tile([P, ET, OWN], BF16, tag="h2_bf")
    emit_layernorm(y1_sb, h2_bf, OWN)

    # ---------------- phase 6: FFN1 + GELU ----------------
    w2_sb = sing.tile([P, MT, E], BF16, tag="w2")
    w2_v = ffn_w2T.rearrange("(o p) e -> p o e", p=P)
    for kt in range(MT):
        nc.sync.dma_start(out=w2_sb[:, kt, :], in_=w2_v[:, kt, :])

    w1_view = ffn_w1T.rearrange("(o p) f -> p o f", p=P)
    ffnh = sing.tile([P, MT, OWN], BF16, tag="ffnh")
    for mt in range(MT):
        wt = w1_pool.tile([P, ET, P], BF16, tag="w1")
        nc.sync.dma_start(out=wt, in_=w1_view[:, :, mt * P:(mt + 1) * P])
        ps = ps_main.tile([P, 512], F32, tag="g")
        for et in range(ET):
            nc.tensor.matmul(ps, wt[:, et, :], h2_bf[:, et, :],
                             start=(et == 0), stop=(et == ET - 1))
        nc.scalar.activation(ffnh[:, mt, :], ps, AF.Gelu,
                             bias=b1_sb[:, mt:mt + 1])

    # ---------------- phase 7: FFN2 + residual + store ----------------
    yT_view = yT.rearrange("(o p) t -> p o t", p=P)
    for et in range(ET):
        ps = ps_main.tile([P, 512], F32, tag="g")
        for ktl in range(MT):
            nc.tensor.matmul(ps, w2_sb[:, ktl, et * P:(et + 1) * P],
                             ffnh[:, ktl, :],
                             start=(ktl == 0), stop=(ktl == MT - 1))
        t = ft_pool.tile([P, 512], F32, tag="ft")
        nc.scalar.activation(t, ps, AF.Identity, bias=b2_sb[:, et:et + 1])
        nc.vector.tensor_add(y1_sb[:, et, :], t, y1_sb[:, et, :])
        nc.sync.dma_start(out=yT_view[:, et, :], in_=y1_sb[:, et, :])

    ctx.close()


# ======================= host side =======================

def prep_inputs(x, ln1_w, ln1_b, qkv_w, qkv_b, out_w, out_b,
                ln2_w, ln2_b, ffn_w1, ffn_b1, ffn_w2, ffn_b2):
    """Shard/fold/cast the full inputs into 8 per-core input maps."""
    x = np.asarray(x, np.float32)
    f8 = lambda v: np.asarray(v, np.float64)

    qkv_wT = (f8(qkv_w) * f8(ln1_w)[None, :]).T.astype(BF16NP).copy()
    qkv_b_eff = (f8(qkv_b) + f8(qkv_w) @ f8(ln1_b)).astype(np.float32)
    out_wT = f8(out_w).T.astype(BF16NP).copy()
    out_b_eff = (f8(out_b) + f8(out_w) @ f8(qkv_b)[2 * E:]).astype(np.float32)
    ffn_w1T = (f8(ffn_w1) * f8(ln2_w)[None, :]).T.astype(BF16NP).copy()
    ffn_b1_eff = (f8(ffn_b1) + f8(ffn_w1) @ f8(ln2_b)).astype(np.float32)
    ffn_w2T = f8(ffn_w2).T.astype(BF16NP).copy()
    ffn_b2_f = np.asarray(ffn_b2, np.float32)

    cidx = np.arange(P)[:, None]   # key (folded, within block)
    ridx = np.arange(P)[None, :]   # query (folded, within block)
    m_prev = (cidx >= ridx).astype(BF16NP)
    m_diag = (cidx <= ridx).astype(BF16NP)
    zero = np.zeros((P, P), BF16NP)

    in_maps = []
    for c in range(N_CORES):
        b, ch = divmod(c, 4)
        lo = OWN * ch - HALO
        if ch == 0:
            slab = np.concatenate(
                [np.zeros((HALO, E), np.float32), x[b, 0:OWN]], axis=0)
        else:
            slab = x[b, lo:lo + SLAB]
        xT = np.ascontiguousarray(slab.T)

        mask = np.stack([
            np.stack([zero if ch == 0 else m_prev, m_diag]),  # qb = 0
            np.stack([m_prev, m_diag]),                       # qb = 1
        ]).astype(BF16NP)

        in_maps.append({
            "xT": xT,
            "qkv_wT": qkv_wT, "out_wT": out_wT,
            "ffn_w1T": ffn_w1T, "ffn_w2T": ffn_w2T,
            "qkv_b": qkv_b_eff, "out_b": out_b_eff,
            "ffn_b1": ffn_b1_eff, "ffn_b2": ffn_b2_f,
            "maskT": np.ascontiguousarray(mask),
        })
    return in_maps


def gather_output(results):
    y = np.empty((B, L, E), np.float32)
    for c in range(N_CORES):
        b, ch = divmod(c, 4)
        y[b, OWN * ch:OWN * (ch + 1)] = results[c]["yT"].T
    return y


_NC_CACHE = None


def _get_program():
    global _NC_CACHE
    if _NC_CACHE is None:
        _NC_CACHE = build_program()
    return _NC_CACHE


def kernel(**inputs):
    nc = _get_program()
    in_maps = prep_inputs(**inputs)
    res = run_bass_kernel_spmd(nc, in_maps, core_ids=list(range(N_CORES)))
    return gather_output(res.results)
